# revision 109
# baseline (speedup 1.0000x reference)
"""nn_CrossAttention Trainium2 kernel — 8-core data-parallel over batch.

Per core (batch slice b=1):
  q1^T / kv1^T via transposed-orientation 1x1 convs (bf16, 2-row PSUM slabs
  evacuated by single DVE/ACT copies into a k/v-interleaved SBUF region);
  depthwise 3x3/7x7 as per-(channel, dh) banded-Toeplitz matmuls on the
  TensorEngine (host-built fp8e3 tiles with per-channel amax scaling —
  q/k scales cancel in the l2 norm, v's fold into the mp rows; PSUM
  accumulation over dh with free-dim h shifts). The q/k/v depthwise streams
  interleave per channel so v tile DMA overlaps k/q PE work; k+v tiles ship
  as one 4-channel wave DMA (p-major contiguous), q as 8-channel waves from
  the Pool queue. l2-norm partials fuse into the writebacks (paired DVE
  square+reduce / ACT square-accum), QK^T with pixels on partitions,
  softmax without max-subtraction (|logits| <= temperature), attn@v merged
  with the output 1x1 conv through a per-pair [96,192] fused matrix; bf16
  output upcast on host.
"""

import sys

sys.path.insert(0, "/opt/trn_rl_repo")

import numpy as np
import ml_dtypes

B, C, Himg, Wimg = 8, 192, 128, 128
HW = Himg * Wimg
HEADS, DHC = 4, 48      # heads, channels per head
PC = 96                 # channels per head-pair
SLAB = 8                # image rows per input stream slab

_PROG = None            # cached (nc, meta)


def _build_toeplitz(wdw, ksz):
    """wdw [c, ksz, ksz] f32 -> ([128, c*ksz, 128] fp8e3 p-major, [c] scales).

    T[w_in, w_out] = wdw[c, dh, w_in - w_out + pad] inside the band, else 0.
    Each channel's ksz tiles share one amax scale so the fp8 mantissa is
    fully used; the p-major [partition, tile, w_out] layout makes each
    DMA wave a single contiguous run per partition.
    """
    pad = ksz // 2
    wi = np.arange(128)[:, None]
    wo = np.arange(128)[None, :]
    idx = wi - wo + pad
    valid = (idx >= 0) & (idx < ksz)
    idxc = np.clip(idx, 0, ksz - 1)
    T = wdw[:, :, idxc] * valid[None, None]          # [c, ksz, 128, 128]
    amax = np.abs(wdw).max(axis=(1, 2))              # [c]
    scale = 14.0 / np.maximum(amax, 1e-12)
    T = T * scale[:, None, None, None]
    T = T.reshape(-1, 128, 128)
    return np.ascontiguousarray(T.transpose(1, 0, 2)), scale


def _split_excess_waits(nc, limit=1):
    """This container's walrus rejects >1 sync wait per instruction (and any
    wait on Drain beyond its own barrier). Hoist extras onto same-engine
    NoOps placed immediately before."""
    import bass_rust
    import concourse.mybir as mybir

    n_split = 0
    for fn in nc.m.functions:
        for bb in fn.blocks:
            insts = bb.instructions
            i = 0
            while i < len(insts):
                inst = insts[i]
                si = inst.sync_info
                lim = 0 if type(inst).__name__ == "InstDrain" else limit
                if si is not None and si.on_wait and len(si.on_wait) > lim:
                    waits = list(si.on_wait)
                    keep, extra = waits[:lim], waits[lim:]
                    pos = i
                    for j in range(0, len(extra), max(limit, 1)):
                        ch = extra[j : j + max(limit, 1)]
                        nop = mybir.InstNoOp(
                            name=f"waitsplit_{n_split}_{pos}",
                            engine=inst.engine,
                            ins=[],
                            outs=[],
                            sync_info=bass_rust.SyncInfo(on_wait=ch, on_update=[]),
                        )
                        insts.insert(pos, nop)
                        pos += 1
                        n_split += 1
                    inst.sync_info = bass_rust.SyncInfo(
                        on_wait=keep, on_update=list(si.on_update)
                    )
                    i = pos + 1
                else:
                    i += 1
    return n_split


def _build_program(split_waits=True):
    import concourse.bass as bass
    import concourse.mybir as mybir
    import concourse.tile as tile

    F32 = mybir.dt.float32
    BF16 = mybir.dt.bfloat16
    FP8 = mybir.dt.float8e3
    AF = mybir.ActivationFunctionType
    OP = mybir.AluOpType

    nc = bass.Bass("TRN2", target_bir_lowering=False, debug=False, num_devices=8)

    # ---- DRAM parameters ----
    xin = nc.dram_tensor("x", [C, HW], BF16, kind="ExternalInput").ap()
    yin = nc.dram_tensor("y", [C, HW], BF16, kind="ExternalInput").ap()
    wqkv_d = nc.dram_tensor("wqkv", [C, C + 384], BF16, kind="ExternalInput").ap()
    wp_d = nc.dram_tensor("wp", [PC, 2 * C], BF16, kind="ExternalInput").ap()
    tq_d = nc.dram_tensor("tq", [128, C * 3, 128], FP8, kind="ExternalInput").ap()
    tkv2_d = nc.dram_tensor("tkv2", [128, 2 * C * 7, 128], FP8, kind="ExternalInput").ap()
    idb_d = nc.dram_tensor("idb", [128, 128], BF16, kind="ExternalInput").ap()
    # maskbd [PC,PC] | vsinv pair [PC,2] | temp*DHC cols [PC,2], one f32 blob
    mv_d = nc.dram_tensor("maskv", [PC, PC + 4], F32, kind="ExternalInput").ap()
    out_d = nc.dram_tensor("out", [C, HW], BF16, kind="ExternalOutput").ap()

    with tile.TileContext(nc) as tc:
        import contextlib

        with contextlib.ExitStack() as ctx:
            consts = ctx.enter_context(tc.tile_pool(name="consts", bufs=1))
            s1 = ctx.enter_context(tc.tile_pool(name="s1", bufs=1))
            s2 = ctx.enter_context(tc.tile_pool(name="s2", bufs=1))
            streams = ctx.enter_context(tc.tile_pool(name="streams", bufs=2))
            tpq = ctx.enter_context(tc.tile_pool(name="tpq", bufs=2))
            tpkv = ctx.enter_context(tc.tile_pool(name="tpkv", bufs=2))
            ps = ctx.enter_context(tc.tile_pool(name="ps", bufs=4, space="PSUM"))
            scratch = ctx.enter_context(tc.tile_pool(name="scratch", bufs=2))
            ostage = ctx.enter_context(tc.tile_pool(name="ostage", bufs=3))
            misc = ctx.enter_context(tc.tile_pool(name="misc", bufs=1))
            stats = ctx.enter_context(tc.tile_pool(name="stats", bufs=1))

            # ---- load constants (packed to minimize DMA count) ----
            wqkv0 = consts.tile([128, C + 384], BF16)
            wqkv1 = consts.tile([64, C + 384], BF16)
            nc.scalar.dma_start(out=wqkv0, in_=wqkv_d[0:128, :])
            nc.scalar.dma_start(out=wqkv1, in_=wqkv_d[128:192, :])
            wq0, wkv0 = wqkv0[:, 0:C], wqkv0[:, C:]
            wq1, wkv1 = wqkv1[:, 0:C], wqkv1[:, C:]
            wpt = consts.tile([PC, 2 * C], BF16)
            nc.scalar.dma_start(out=wpt, in_=wp_d)
            wp0, wp1 = wpt[:, 0:C], wpt[:, C:]
            identb = consts.tile([128, 128], BF16)
            nc.scalar.dma_start(out=identb, in_=idb_d)
            maskv = consts.tile([PC, PC + 4], F32)
            nc.scalar.dma_start(out=maskv, in_=mv_d)
            maskbd = maskv[:, 0:PC]
            vsinv0 = maskv[:, PC : PC + 1]
            vsinv1 = maskv[:, PC + 1 : PC + 2]
            tempc0 = maskv[:, PC + 2 : PC + 3]
            tempc1 = maskv[:, PC + 3 : PC + 4]
            onescol = consts.tile([128, 1], BF16)
            nc.vector.memset(onescol, 1.0)

            # ---- big SBUF regions ----
            # bq: [w partitions, h*192 + c]; bkv: [w, h*384 + c] with k at
            # columns h*384+c and v at h*384+192+c (k/v interleaved per h so
            # the 1x1-conv PSUM slab evacuates in one copy per row block).
            bq = s1.tile([128, Himg * C], BF16, tag="qv")
            bkv = s2.tile([128, Himg * 2 * C], BF16, tag="kv")
            bq3 = bq.rearrange("p (h c) -> p h c", c=C)
            bkv3 = bkv.rearrange("p (h c) -> p h c", c=2 * C)

            def chan_ap(region3, c, col0, cnt):
                # [128, cnt] strided view: channel c, h-columns col0..col0+cnt
                return region3[:, col0 : col0 + cnt, c]

            # ================= Phase A: 1x1 convs (transposed orientation) ==
            # 2 image rows accumulate into one PSUM slab; a single DVE copy
            # evacuates the slab into the (interleaved) SBUF region. The
            # slab pool's PSUM banks are scoped to this phase.
            psA_cm = tc.tile_pool(name="psA", bufs=4, space="PSUM")
            psA = psA_cm.__enter__()

            def conv1x1_phase(src_d, mov0, mov1, nmov, dst, HB):
                pt = None
                for h in range(Himg):
                    sl = h % SLAB
                    if sl == 0:
                        xs0 = streams.tile([128, SLAB * 128], BF16, tag="st0")
                        xs1 = streams.tile([64, SLAB * 128], BF16, tag="st1")
                        nc.sync.dma_start(
                            out=xs0, in_=src_d[0:128, h * 128 : (h + SLAB) * 128]
                        )
                        nc.sync.dma_start(
                            out=xs1, in_=src_d[128:192, h * 128 : (h + SLAB) * 128]
                        )
                    hb = h % HB
                    if hb == 0:
                        pt = psA.tile([128, HB * nmov], F32, tag="psA")
                    sub = pt[:, hb * nmov : (hb + 1) * nmov]
                    nc.tensor.matmul(
                        sub, xs0[:, sl * 128 : (sl + 1) * 128], mov0,
                        start=True, stop=False,
                    )
                    nc.tensor.matmul(
                        sub, xs1[:, sl * 128 : (sl + 1) * 128], mov1,
                        start=False, stop=True,
                    )
                    if hb == HB - 1:
                        h0 = h - HB + 1
                        dslice = dst[:, h0 * nmov : (h0 + HB) * nmov]
                        if (h // HB) % 2 == 0:
                            nc.vector.tensor_copy(dslice, pt)
                        else:
                            nc.scalar.activation(out=dslice, in_=pt, func=AF.Copy)

            conv1x1_phase(xin, wq0, wq1, C, bq, 2)
            conv1x1_phase(yin, wkv0, wkv1, 2 * C, bkv, 1)
            psA_cm.__exit__(None, None, None)
            attnp_cm = tc.tile_pool(name="attnp", bufs=2, space="PSUM")
            attnp_pool = attnp_cm.__enter__()

            # ====== Phase B: merged q/k/v depthwise via Toeplitz matmuls ====
            # The three streams interleave per channel so v's bf16 tile DMA
            # overlaps the k/q PE work. partials[:, t*C + ci] = per-partition
            # sum of squares fused into the writebacks (q: t=0, k: t=1).
            partials = stats.tile([128, 2 * C], F32)
            partials_bf = stats.tile([128, 2 * C], BF16)

            # channel-pair transposed views for the paired sum-of-squares
            bq_ch = bq.rearrange("p (h c) -> p c h", c=C)
            bkv_ch = bkv.rearrange("p (h c) -> p c h", c=2 * C)

            def square_quad(ch_view, c0, pcols):
                # one DVE square + reduce for two adjacent dw channels;
                # bf16 scratch halves the DVE write/read traffic
                quad = ch_view[:, c0 : c0 + 2, :]
                sc = scratch.tile([128, 2, 128], BF16, tag="sqf")
                nc.vector.tensor_tensor(sc, quad, quad, op=OP.mult)
                nc.vector.tensor_reduce(
                    pcols, sc, axis=mybir.AxisListType.X, op=OP.add
                )

            def dw_channel(tw, base, ksz, region3, cidx):
                pad = ksz // 2
                order = [pad] + [d for d in range(ksz) if d != pad]
                pdw = ps.tile([128, 128], F32, tag="ps")
                for j, dh in enumerate(order):
                    sh = dh - pad
                    cnt = Himg - abs(sh)
                    h0o, h0i = max(0, -sh), max(0, sh)
                    nc.tensor.matmul(
                        pdw[:, h0o : h0o + cnt],
                        tw[:, base + dh, :],
                        chan_ap(region3, cidx, h0i, cnt),
                        start=(j == 0),
                        stop=(j == len(order) - 1),
                    )
                return pdw

            CWKV, CWQ = 4, 8
            twkv = twq = None
            for ci in range(C):
                cb = ci % CWKV
                # --- k+v share one fp8 wave DMA per 4 channels (SP) ---
                if cb == 0:
                    twkv = tpkv.tile([128, CWKV * 14, 128], FP8, tag="tw",
                                     name=f"tw_kv_{ci}")
                    i0 = ci * 14
                    nc.sync.dma_start(
                        out=twkv, in_=tkv2_d[:, i0 : i0 + CWKV * 14, :]
                    )
                # --- k: ACT evacuates; paired squares on DVE ---
                pdw = dw_channel(twkv, cb * 7, 7, bkv3, ci)
                kdst = chan_ap(bkv3, ci, 0, Himg)
                nc.scalar.activation(out=kdst, in_=pdw, func=AF.Copy)
                if ci % 2 == 1:
                    square_quad(bkv_ch, ci - 1, partials[:, C + ci - 1 : C + ci + 1])
                # --- v: ACT evacuates ---
                pdw = dw_channel(twkv, CWKV * 7 + cb * 7, 7, bkv3, C + ci)
                nc.scalar.activation(
                    out=chan_ap(bkv3, C + ci, 0, Himg), in_=pdw, func=AF.Copy
                )
                # --- q (fp8, 8-channel waves issued from the Pool queue) ---
                if ci % CWQ == 0:
                    twq = tpq.tile([128, CWQ * 3, 128], FP8, tag="tw",
                                   name=f"tw_q_{ci}")
                    i0 = ci * 3
                    nc.gpsimd.dma_start(
                        out=twq, in_=tq_d[:, i0 : i0 + CWQ * 3, :]
                    )
                pdw = dw_channel(twq, (ci % CWQ) * 3, 3, bq3, ci)
                qdst = chan_ap(bq3, ci, 0, Himg)
                nc.vector.tensor_copy(qdst, pdw)
                if ci % 2 == 1:
                    square_quad(bq_ch, ci - 1, partials[:, ci - 1 : ci + 1])
            nc.vector.tensor_copy(partials_bf, partials)

            # ================= Phase D: QK^T + softmax prep per pair ========
            # QKT accumulations first; the small softmax-prep chains are
            # emitted between the halves of the v depthwise so PE never
            # waits on the serial DVE/ACT chains.
            attnps = []
            for P in range(2):
                attnp = attnp_pool.tile([128, PC], F32, tag="at")
                for h in range(Himg):
                    nc.tensor.matmul(
                        attnp,
                        bkv[:, h * 2 * C + PC * P : h * 2 * C + PC * P + 128],
                        bq[:, h * C + PC * P : h * C + PC * P + PC],
                        start=(h == 0),
                        stop=(h == Himg - 1),
                    )
                attnps.append(attnp)

            # ===== Phase F+G: softmax prep, then fused v-transpose + proj ===
            # l2-norm scales fold in as per-partition scalars on both sides
            # of ONE transpose: rk multiplies attnp while k is on partitions,
            # rq (with temperature) rides the Exp scale after transposing to
            # [q-ch, k-ch] — which is exactly the orientation the fused
            # (attn@v)+proj matmul wants, so the old ezt transpose vanishes.
            pcols = []
            for sl in (0, PC, C, C + PC):        # q P0, q P1, k P0, k P1
                pcol = ps.tile([PC, 1], F32, tag="ps")
                nc.tensor.matmul(
                    pcol, partials_bf[:, sl : sl + PC], onescol,
                    start=True, stop=True,
                )
                pcols.append(pcol)
            rqs, rks = [], []
            for i in range(4):
                sq = misc.tile([PC, 1], F32, tag=f"sq{i}")
                nc.scalar.activation(out=sq, in_=pcols[i], func=AF.Sqrt)
                r = misc.tile([PC, 1], F32, tag=f"r{i}")
                nc.vector.reciprocal(r, sq)
                (rqs if i < 2 else rks).append(r)
            for P in range(2):
                nc.vector.tensor_tensor(
                    rqs[P], rqs[P], (tempc0, tempc1)[P], op=OP.mult
                )
            sb1s = []
            for P in range(2):
                sb1 = misc.tile([PC, PC], BF16, tag=f"sb{P}")
                nc.vector.tensor_scalar_mul(sb1, attnps[P][0:PC, :], rks[P])
                sb1s.append(sb1)
            attnp_cm.__exit__(None, None, None)
            pst = ctx.enter_context(tc.tile_pool(name="pst", bufs=3, space="PSUM"))
            vtpool = ctx.enter_context(tc.tile_pool(name="vtw", bufs=2))

            mps = []
            for P in range(2):
                etp = pst.tile([PC, PC], BF16, tag="tp")
                nc.tensor.transpose(etp, sb1s[P], identb[0:PC, 0:PC])
                e_t = misc.tile([PC, PC], F32, tag=f"et{P}")
                nc.scalar.activation(out=e_t, in_=etp, func=AF.Exp, scale=rqs[P])
                ez = stats.tile([PC, PC], BF16, tag=f"ez{P}")
                nc.vector.tensor_tensor(ez, e_t, maskbd, op=OP.mult)
                dsum = misc.tile([PC, 1], F32, tag=f"ds{P}")
                nc.vector.tensor_reduce(
                    dsum, ez, axis=mybir.AxisListType.X, op=OP.add
                )
                recip = stats.tile([PC, 1], F32, tag=f"rc{P}")
                nc.vector.reciprocal(recip, dsum)
                wsc = misc.tile([PC, C], BF16, tag=f"m10{P}")
                nc.vector.tensor_scalar_mul(wsc, (wp0, wp1)[P], recip)
                pmp = ps.tile([PC, C], F32, tag="ps")
                nc.tensor.matmul(pmp, ez, wsc, start=True, stop=True)
                mp = stats.tile([PC, C], BF16, tag=f"mp{P}")
                nc.vector.tensor_scalar_mul(mp, pmp, (vsinv0, vsinv1)[P])
                mps.append(mp)

            # per 8 image rows: transpose both v pair-slices into a rolling
            # [PC, 2, 1024] window, then immediately project + store. No
            # full-vt barrier, and only ~4KB of vt SBUF live at once.
            for hb in range(0, Himg, 8):
                vtw = vtpool.tile([PC, 2, 1024], BF16, tag="vw")
                for P in range(2):
                    for jb in (0, 4):
                        ptv = pst.tile([PC, 512], BF16, tag="tp")
                        for j in range(4):
                            h = hb + jb + j
                            base = h * 2 * C + C + PC * P
                            nc.tensor.transpose(
                                ptv[:, j * 128 : (j + 1) * 128],
                                bkv[:, base : base + PC],
                                identb,
                            )
                        dst = vtw[:, P, jb * 128 : jb * 128 + 512]
                        if (P + jb // 4) % 2 == 0:
                            nc.vector.tensor_copy(dst, ptv)
                        else:
                            nc.scalar.activation(out=dst, in_=ptv, func=AF.Copy)
                n = hb * 128
                for r0, r1 in ((0, 128), (128, 192)):
                    mw = r1 - r0
                    so = ostage.tile([mw, 1024], BF16, tag=f"os{r0}")
                    for half in range(2):
                        po = ps.tile([mw, 512], F32, tag="ps")
                        nc.tensor.matmul(
                            po, mps[0][:, r0:r1],
                            vtw[:, 0, half * 512 : (half + 1) * 512],
                            start=True, stop=False,
                        )
                        nc.tensor.matmul(
                            po, mps[1][:, r0:r1],
                            vtw[:, 1, half * 512 : (half + 1) * 512],
                            start=False, stop=True,
                        )
                        dst = so[:, half * 512 : (half + 1) * 512]
                        if half == 0:
                            nc.vector.tensor_copy(dst, po)
                        else:
                            nc.scalar.activation(out=dst, in_=po, func=AF.Copy)
                    nc.sync.dma_start(out=out_d[r0:r1, n : n + 1024], in_=so)

    if split_waits:
        _split_excess_waits(nc)
    return nc


def _get_program():
    global _PROG
    if _PROG is None:
        _PROG = _build_program()
    return _PROG


def kernel(x, y, q_w, q_dw_w, kv_w, kv_dw_w, proj_w, temperature):
    return _run(x, y, q_w, q_dw_w, kv_w, kv_dw_w, proj_w, temperature)[0]


def _prepare_inputs(x, y, q_w, q_dw_w, kv_w, kv_dw_w, proj_w, temperature):
    x = np.asarray(x, dtype=np.float32).reshape(B, C, HW).astype(ml_dtypes.bfloat16)
    y = np.asarray(y, dtype=np.float32).reshape(B, C, HW).astype(ml_dtypes.bfloat16)
    q_w = np.asarray(q_w, dtype=np.float32)
    kv_w = np.asarray(kv_w, dtype=np.float32)
    proj_w = np.asarray(proj_w, dtype=np.float32)
    q_dw_w = np.asarray(q_dw_w, dtype=np.float32)
    kv_dw_w = np.asarray(kv_dw_w, dtype=np.float32)
    temperature = np.asarray(temperature, dtype=np.float32).reshape(HEADS)

    tq, _ = _build_toeplitz(q_dw_w[:, 0], 3)
    tq = tq.astype(ml_dtypes.float8_e3m4)
    tkv, skv = _build_toeplitz(kv_dw_w[:, 0], 7)
    tkv = tkv.astype(ml_dtypes.float8_e3m4)
    # interleave k and v tiles into 4-channel wave blocks:
    # block g = [k(4g..4g+3) x7 tiles | v(4g..4g+3) x7 tiles]
    tk = tkv[:, : C * 7].reshape(128, C // 4, 28, 128)
    tv = tkv[:, C * 7 :].reshape(128, C // 4, 28, 128)
    tkv2 = np.ascontiguousarray(
        np.concatenate([tk, tv], axis=2).reshape(128, 2 * C * 7, 128)
    )

    wq = q_w[:, :, 0, 0].T
    wkv = kv_w[:, :, 0, 0].T
    wqkv = np.ascontiguousarray(
        np.concatenate([wq, wkv], axis=1)
    ).astype(ml_dtypes.bfloat16)                            # [C, C+384]
    wpT = proj_w[:, :, 0, 0].T                              # [c_in, c_out]
    wp = np.ascontiguousarray(
        np.concatenate([wpT[0:PC], wpT[PC:C]], axis=1)
    ).astype(ml_dtypes.bfloat16)                            # [PC, 2C]
    # undo the per-channel fp8 amax scaling of v's depthwise weights on the
    # mp rows (indexed by v-channel d); q/k scales cancel in the l2 norm
    vsinv = (1.0 / skv[C:]).reshape(2, PC).T.astype(np.float32)  # [PC, 2]
    idb = np.eye(128, dtype=ml_dtypes.bfloat16)
    maskbd = np.zeros((PC, PC), np.float32)
    maskbd[0:DHC, 0:DHC] = 1.0
    maskbd[DHC:PC, DHC:PC] = 1.0
    tempcol = np.repeat(temperature, DHC).reshape(2, PC).T  # [PC, 2]
    maskv = np.ascontiguousarray(
        np.concatenate([maskbd, vsinv, tempcol], axis=1)
    ).astype(np.float32)

    shared = {
        "wqkv": wqkv, "wp": wp, "tq": tq, "tkv2": tkv2,
        "idb": idb, "maskv": maskv,
    }
    return [dict(shared, x=x[i], y=y[i]) for i in range(B)]


def _postprocess(out_core):
    return np.asarray(out_core).reshape(C, Himg, Wimg).astype(np.float32)


def _run(x, y, q_w, q_dw_w, kv_w, kv_dw_w, proj_w, temperature, trace=False):
    from concourse.bass_utils import run_bass_kernel_spmd

    in_maps = _prepare_inputs(
        x, y, q_w, q_dw_w, kv_w, kv_dw_w, proj_w, temperature
    )
    nc = _get_program()
    res = run_bass_kernel_spmd(
        nc, in_maps, core_ids=list(range(B)), trace=trace
    )
    out = np.stack([_postprocess(res.results[i]["out"]) for i in range(B)])
    return out, res



# revision 115
# speedup vs baseline: 1.0186x; 1.0186x over previous
"""nn_CrossAttention Trainium2 kernel — 8-core data-parallel over batch.

Per core (batch slice b=1):
  q1^T / kv1^T via transposed-orientation 1x1 convs (bf16, 2-row PSUM slabs
  evacuated by single DVE/ACT copies into a k/v-interleaved SBUF region);
  depthwise 3x3/7x7 as per-(channel, dh) banded-Toeplitz matmuls on the
  TensorEngine (host-built fp8e3 tiles with per-channel amax scaling —
  q/k scales cancel in the l2 norm, v's fold into the mp rows; PSUM
  accumulation over dh with free-dim h shifts). The q/k/v depthwise streams
  interleave per channel so v tile DMA overlaps k/q PE work; k+v tiles ship
  as one 4-channel wave DMA (p-major contiguous), q as 8-channel waves from
  the Pool queue. l2-norm partials fuse into the writebacks (paired DVE
  square+reduce / ACT square-accum), QK^T with pixels on partitions,
  softmax without max-subtraction (|logits| <= temperature), attn@v merged
  with the output 1x1 conv through a per-pair [96,192] fused matrix; bf16
  output upcast on host.
"""

import sys

sys.path.insert(0, "/opt/trn_rl_repo")

import numpy as np
import ml_dtypes

B, C, Himg, Wimg = 8, 192, 128, 128
HW = Himg * Wimg
HEADS, DHC = 4, 48      # heads, channels per head
PC = 96                 # channels per head-pair
SLAB = 8                # image rows per input stream slab

_PROG = None            # cached (nc, meta)


def _build_toeplitz(wdw, ksz):
    """wdw [c, ksz, ksz] f32 -> ([128, c*ksz, 128] fp8e3 p-major, [c] scales).

    T[w_in, w_out] = wdw[c, dh, w_in - w_out + pad] inside the band, else 0.
    Each channel's ksz tiles share one amax scale so the fp8 mantissa is
    fully used; the p-major [partition, tile, w_out] layout makes each
    DMA wave a single contiguous run per partition.
    """
    pad = ksz // 2
    wi = np.arange(128)[:, None]
    wo = np.arange(128)[None, :]
    idx = wi - wo + pad
    valid = (idx >= 0) & (idx < ksz)
    idxc = np.clip(idx, 0, ksz - 1)
    T = wdw[:, :, idxc] * valid[None, None]          # [c, ksz, 128, 128]
    amax = np.abs(wdw).max(axis=(1, 2))              # [c]
    scale = 14.0 / np.maximum(amax, 1e-12)
    T = T * scale[:, None, None, None]
    T = T.reshape(-1, 128, 128)
    return np.ascontiguousarray(T.transpose(1, 0, 2)), scale


def _split_excess_waits(nc, limit=1):
    """This container's walrus rejects >1 sync wait per instruction (and any
    wait on Drain beyond its own barrier). Hoist extras onto same-engine
    NoOps placed immediately before."""
    import bass_rust
    import concourse.mybir as mybir

    n_split = 0
    for fn in nc.m.functions:
        for bb in fn.blocks:
            insts = bb.instructions
            i = 0
            while i < len(insts):
                inst = insts[i]
                si = inst.sync_info
                lim = 0 if type(inst).__name__ == "InstDrain" else limit
                if si is not None and si.on_wait and len(si.on_wait) > lim:
                    waits = list(si.on_wait)
                    keep, extra = waits[:lim], waits[lim:]
                    pos = i
                    for j in range(0, len(extra), max(limit, 1)):
                        ch = extra[j : j + max(limit, 1)]
                        nop = mybir.InstNoOp(
                            name=f"waitsplit_{n_split}_{pos}",
                            engine=inst.engine,
                            ins=[],
                            outs=[],
                            sync_info=bass_rust.SyncInfo(on_wait=ch, on_update=[]),
                        )
                        insts.insert(pos, nop)
                        pos += 1
                        n_split += 1
                    inst.sync_info = bass_rust.SyncInfo(
                        on_wait=keep, on_update=list(si.on_update)
                    )
                    i = pos + 1
                else:
                    i += 1
    return n_split


def _build_program(split_waits=True):
    import concourse.bass as bass
    import concourse.mybir as mybir
    import concourse.tile as tile

    F32 = mybir.dt.float32
    BF16 = mybir.dt.bfloat16
    FP8 = mybir.dt.float8e3
    AF = mybir.ActivationFunctionType
    OP = mybir.AluOpType

    nc = bass.Bass("TRN2", target_bir_lowering=False, debug=False, num_devices=8)

    # ---- DRAM parameters ----
    xin = nc.dram_tensor("x", [128, HW + HW // 2], BF16, kind="ExternalInput").ap()
    yin = nc.dram_tensor("y", [128, HW + HW // 2], BF16, kind="ExternalInput").ap()
    wqkv_d = nc.dram_tensor("wqkv", [256, C + 384], BF16, kind="ExternalInput").ap()
    wp_d = nc.dram_tensor("wp", [PC, 2 * C], BF16, kind="ExternalInput").ap()
    tq_d = nc.dram_tensor("tq", [128, C * 3, 128], FP8, kind="ExternalInput").ap()
    tkv2_d = nc.dram_tensor("tkv2", [128, 2 * C * 7, 128], FP8, kind="ExternalInput").ap()
    idb_d = nc.dram_tensor("idb", [128, 128], BF16, kind="ExternalInput").ap()
    # maskbd [PC,PC] | vsinv pair [PC,2] | temp*DHC cols [PC,2], one f32 blob
    mv_d = nc.dram_tensor("maskv", [PC, PC + 4], F32, kind="ExternalInput").ap()
    out_d = nc.dram_tensor("out", [C, HW], BF16, kind="ExternalOutput").ap()

    with tile.TileContext(nc) as tc:
        import contextlib

        with contextlib.ExitStack() as ctx:
            consts = ctx.enter_context(tc.tile_pool(name="consts", bufs=1))
            s1 = ctx.enter_context(tc.tile_pool(name="s1", bufs=1))
            s2 = ctx.enter_context(tc.tile_pool(name="s2", bufs=1))
            streams = ctx.enter_context(tc.tile_pool(name="streams", bufs=2))
            tpq = ctx.enter_context(tc.tile_pool(name="tpq", bufs=2))
            tpkv = ctx.enter_context(tc.tile_pool(name="tpkv", bufs=2))
            ps = ctx.enter_context(tc.tile_pool(name="ps", bufs=4, space="PSUM"))
            scratch = ctx.enter_context(tc.tile_pool(name="scratch", bufs=2))
            ostage = ctx.enter_context(tc.tile_pool(name="ostage", bufs=3))
            misc = ctx.enter_context(tc.tile_pool(name="misc", bufs=1))
            stats = ctx.enter_context(tc.tile_pool(name="stats", bufs=1))

            # ---- load constants (packed to minimize DMA count) ----
            wqkv0 = consts.tile([128, C + 384], BF16)
            wqkv1 = consts.tile([128, C + 384], BF16)
            nc.scalar.dma_start(out=wqkv0, in_=wqkv_d[0:128, :])
            nc.scalar.dma_start(out=wqkv1, in_=wqkv_d[128:256, :])
            wq0, wkv0 = wqkv0[:, 0:C], wqkv0[:, C:]
            wq1, wkv1 = wqkv1[:, 0:C], wqkv1[:, C:]
            wpt = consts.tile([PC, 2 * C], BF16)
            nc.scalar.dma_start(out=wpt, in_=wp_d)
            wp0, wp1 = wpt[:, 0:C], wpt[:, C:]
            identb = consts.tile([128, 128], BF16)
            nc.scalar.dma_start(out=identb, in_=idb_d)
            maskv = consts.tile([PC, PC + 4], F32)
            nc.scalar.dma_start(out=maskv, in_=mv_d)
            maskbd = maskv[:, 0:PC]
            vsinv0 = maskv[:, PC : PC + 1]
            vsinv1 = maskv[:, PC + 1 : PC + 2]
            tempc0 = maskv[:, PC + 2 : PC + 3]
            tempc1 = maskv[:, PC + 3 : PC + 4]
            onescol = consts.tile([128, 1], BF16)
            nc.vector.memset(onescol, 1.0)

            # ---- big SBUF regions ----
            # bq: [w partitions, h*192 + c]; bkv: [w, h*384 + c] with k at
            # columns h*384+c and v at h*384+192+c (k/v interleaved per h so
            # the 1x1-conv PSUM slab evacuates in one copy per row block).
            bq = s1.tile([128, Himg * C], BF16, tag="qv")
            bkv = s2.tile([128, Himg * 2 * C], BF16, tag="kv")
            bq3 = bq.rearrange("p (h c) -> p h c", c=C)
            bkv3 = bkv.rearrange("p (h c) -> p h c", c=2 * C)

            def chan_ap(region3, c, col0, cnt):
                # [128, cnt] strided view: channel c, h-columns col0..col0+cnt
                return region3[:, col0 : col0 + cnt, c]

            # ================= Phase A: 1x1 convs (transposed orientation) ==
            # 2 image rows accumulate into one PSUM slab; a single DVE copy
            # evacuates the slab into the (interleaved) SBUF region. The
            # slab pool's PSUM banks are scoped to this phase.
            psA_cm = tc.tile_pool(name="psA", bufs=4, space="PSUM")
            psA = psA_cm.__enter__()

            def conv1x1_phase(src_d, mov0, mov1, nmov, dst, HB):
                pt = None
                xs = None
                for h in range(Himg):
                    sl = h % SLAB
                    if sl == 0:
                        xs = streams.tile([128, SLAB * 192], BF16, tag="st0")
                        s = h // SLAB
                        nc.sync.dma_start(
                            out=xs, in_=src_d[:, s * 1536 : (s + 1) * 1536]
                        )
                    hb = h % HB
                    if hb == 0:
                        pt = psA.tile([128, HB * nmov], F32, tag="psA")
                    sub = pt[:, hb * nmov : (hb + 1) * nmov]
                    nc.tensor.matmul(
                        sub, xs[:, sl * 128 : (sl + 1) * 128], mov0,
                        start=True, stop=False,
                    )
                    # channels 128..191 of this row live in the packed tail,
                    # halves stacked on partitions 0:64 / 64:128
                    if sl < SLAB // 2:
                        st2 = xs[0:64, 1024 + sl * 128 : 1024 + (sl + 1) * 128]
                        mv2 = mov1[0:64, :]
                    else:
                        sl2 = sl - SLAB // 2
                        st2 = xs[64:128, 1024 + sl2 * 128 : 1024 + (sl2 + 1) * 128]
                        mv2 = mov1[64:128, :]
                    nc.tensor.matmul(sub, st2, mv2, start=False, stop=True)
                    if hb == HB - 1:
                        h0 = h - HB + 1
                        dslice = dst[:, h0 * nmov : (h0 + HB) * nmov]
                        if (h // HB) % 2 == 0:
                            nc.vector.tensor_copy(dslice, pt)
                        else:
                            nc.scalar.activation(out=dslice, in_=pt, func=AF.Copy)

            conv1x1_phase(xin, wq0, wq1, C, bq, 2)
            conv1x1_phase(yin, wkv0, wkv1, 2 * C, bkv, 1)
            psA_cm.__exit__(None, None, None)
            attnp_cm = tc.tile_pool(name="attnp", bufs=2, space="PSUM")
            attnp_pool = attnp_cm.__enter__()

            # ====== Phase B: merged q/k/v depthwise via Toeplitz matmuls ====
            # The three streams interleave per channel so v's bf16 tile DMA
            # overlaps the k/q PE work. partials[:, t*C + ci] = per-partition
            # sum of squares fused into the writebacks (q: t=0, k: t=1).
            partials = stats.tile([128, 2 * C], F32)
            partials_bf = stats.tile([128, 2 * C], BF16)

            # channel-pair transposed views for the paired sum-of-squares
            bq_ch = bq.rearrange("p (h c) -> p c h", c=C)
            bkv_ch = bkv.rearrange("p (h c) -> p c h", c=2 * C)

            def square_quad(ch_view, c0, pcols):
                # one DVE square + reduce for two adjacent dw channels;
                # bf16 scratch halves the DVE write/read traffic
                quad = ch_view[:, c0 : c0 + 2, :]
                sc = scratch.tile([128, 2, 128], BF16, tag="sqf")
                nc.vector.tensor_tensor(sc, quad, quad, op=OP.mult)
                nc.vector.tensor_reduce(
                    pcols, sc, axis=mybir.AxisListType.X, op=OP.add
                )

            def dw_channel(tw, base, ksz, region3, cidx):
                pad = ksz // 2
                order = [pad] + [d for d in range(ksz) if d != pad]
                pdw = ps.tile([128, 128], F32, tag="ps")
                for j, dh in enumerate(order):
                    sh = dh - pad
                    cnt = Himg - abs(sh)
                    h0o, h0i = max(0, -sh), max(0, sh)
                    nc.tensor.matmul(
                        pdw[:, h0o : h0o + cnt],
                        tw[:, base + dh, :],
                        chan_ap(region3, cidx, h0i, cnt),
                        start=(j == 0),
                        stop=(j == len(order) - 1),
                    )
                return pdw

            CWKV, CWQ = 4, 8
            twkv = twq = None
            for ci in range(C):
                cb = ci % CWKV
                # --- k+v share one fp8 wave DMA per 4 channels (SP) ---
                if cb == 0:
                    twkv = tpkv.tile([128, CWKV * 14, 128], FP8, tag="tw",
                                     name=f"tw_kv_{ci}")
                    i0 = ci * 14
                    nc.sync.dma_start(
                        out=twkv, in_=tkv2_d[:, i0 : i0 + CWKV * 14, :]
                    )
                # --- k: ACT evacuates; paired squares on DVE ---
                pdw = dw_channel(twkv, cb * 7, 7, bkv3, ci)
                kdst = chan_ap(bkv3, ci, 0, Himg)
                nc.scalar.activation(out=kdst, in_=pdw, func=AF.Copy)
                if ci % 2 == 1:
                    square_quad(bkv_ch, ci - 1, partials[:, C + ci - 1 : C + ci + 1])
                # --- v: ACT evacuates ---
                pdw = dw_channel(twkv, CWKV * 7 + cb * 7, 7, bkv3, C + ci)
                nc.scalar.activation(
                    out=chan_ap(bkv3, C + ci, 0, Himg), in_=pdw, func=AF.Copy
                )
                # --- q (fp8, 8-channel waves issued from the Pool queue) ---
                if ci % CWQ == 0:
                    twq = tpq.tile([128, CWQ * 3, 128], FP8, tag="tw",
                                   name=f"tw_q_{ci}")
                    i0 = ci * 3
                    nc.gpsimd.dma_start(
                        out=twq, in_=tq_d[:, i0 : i0 + CWQ * 3, :]
                    )
                pdw = dw_channel(twq, (ci % CWQ) * 3, 3, bq3, ci)
                qdst = chan_ap(bq3, ci, 0, Himg)
                nc.vector.tensor_copy(qdst, pdw)
                if ci % 2 == 1:
                    square_quad(bq_ch, ci - 1, partials[:, ci - 1 : ci + 1])
            nc.vector.tensor_copy(partials_bf, partials)

            # ================= Phase D: QK^T + softmax prep per pair ========
            # QKT accumulations first; the small softmax-prep chains are
            # emitted between the halves of the v depthwise so PE never
            # waits on the serial DVE/ACT chains.
            attnps = []
            for P in range(2):
                attnp = attnp_pool.tile([128, PC], F32, tag="at")
                for h in range(Himg):
                    nc.tensor.matmul(
                        attnp,
                        bkv[:, h * 2 * C + PC * P : h * 2 * C + PC * P + 128],
                        bq[:, h * C + PC * P : h * C + PC * P + PC],
                        start=(h == 0),
                        stop=(h == Himg - 1),
                    )
                attnps.append(attnp)

            # ===== Phase F+G: softmax prep, then fused v-transpose + proj ===
            # l2-norm scales fold in as per-partition scalars on both sides
            # of ONE transpose: rk multiplies attnp while k is on partitions,
            # rq (with temperature) rides the Exp scale after transposing to
            # [q-ch, k-ch] — which is exactly the orientation the fused
            # (attn@v)+proj matmul wants, so the old ezt transpose vanishes.
            pcols = []
            for sl in (0, PC, C, C + PC):        # q P0, q P1, k P0, k P1
                pcol = ps.tile([PC, 1], F32, tag="ps")
                nc.tensor.matmul(
                    pcol, partials_bf[:, sl : sl + PC], onescol,
                    start=True, stop=True,
                )
                pcols.append(pcol)
            rqs, rks = [], []
            for i in range(4):
                sq = misc.tile([PC, 1], F32, tag=f"sq{i}")
                nc.scalar.activation(out=sq, in_=pcols[i], func=AF.Sqrt)
                r = misc.tile([PC, 1], F32, tag=f"r{i}")
                nc.vector.reciprocal(r, sq)
                (rqs if i < 2 else rks).append(r)
            for P in range(2):
                nc.vector.tensor_tensor(
                    rqs[P], rqs[P], (tempc0, tempc1)[P], op=OP.mult
                )
            sb1s = []
            for P in range(2):
                sb1 = misc.tile([PC, PC], BF16, tag=f"sb{P}")
                nc.vector.tensor_scalar_mul(sb1, attnps[P][0:PC, :], rks[P])
                sb1s.append(sb1)
            attnp_cm.__exit__(None, None, None)
            pst = ctx.enter_context(tc.tile_pool(name="pst", bufs=3, space="PSUM"))
            vtpool = ctx.enter_context(tc.tile_pool(name="vtw", bufs=2))

            mps = []
            for P in range(2):
                etp = pst.tile([PC, PC], BF16, tag="tp")
                nc.tensor.transpose(etp, sb1s[P], identb[0:PC, 0:PC])
                e_t = misc.tile([PC, PC], F32, tag=f"et{P}")
                nc.scalar.activation(out=e_t, in_=etp, func=AF.Exp, scale=rqs[P])
                ez = stats.tile([PC, PC], BF16, tag=f"ez{P}")
                nc.vector.tensor_tensor(ez, e_t, maskbd, op=OP.mult)
                dsum = misc.tile([PC, 1], F32, tag=f"ds{P}")
                nc.vector.tensor_reduce(
                    dsum, ez, axis=mybir.AxisListType.X, op=OP.add
                )
                recip = stats.tile([PC, 1], F32, tag=f"rc{P}")
                nc.vector.reciprocal(recip, dsum)
                wsc = misc.tile([PC, C], BF16, tag=f"m10{P}")
                nc.vector.tensor_scalar_mul(wsc, (wp0, wp1)[P], recip)
                pmp = ps.tile([PC, C], F32, tag="ps")
                nc.tensor.matmul(pmp, ez, wsc, start=True, stop=True)
                mp = stats.tile([PC, C], BF16, tag=f"mp{P}")
                nc.vector.tensor_scalar_mul(mp, pmp, (vsinv0, vsinv1)[P])
                mps.append(mp)

            # per 8 image rows: transpose both v pair-slices into a rolling
            # [PC, 2, 1024] window, then immediately project + store. No
            # full-vt barrier, and only ~4KB of vt SBUF live at once.
            for hb in range(0, Himg, 8):
                vtw = vtpool.tile([PC, 2, 1024], BF16, tag="vw")
                for P in range(2):
                    for jb in (0, 4):
                        ptv = pst.tile([PC, 512], BF16, tag="tp")
                        for j in range(4):
                            h = hb + jb + j
                            base = h * 2 * C + C + PC * P
                            nc.tensor.transpose(
                                ptv[:, j * 128 : (j + 1) * 128],
                                bkv[:, base : base + PC],
                                identb,
                            )
                        dst = vtw[:, P, jb * 128 : jb * 128 + 512]
                        if (P + jb // 4) % 2 == 0:
                            nc.vector.tensor_copy(dst, ptv)
                        else:
                            nc.scalar.activation(out=dst, in_=ptv, func=AF.Copy)
                n = hb * 128
                for r0, r1 in ((0, 128), (128, 192)):
                    mw = r1 - r0
                    so = ostage.tile([mw, 1024], BF16, tag=f"os{r0}")
                    for half in range(2):
                        po = ps.tile([mw, 512], F32, tag="ps")
                        nc.tensor.matmul(
                            po, mps[0][:, r0:r1],
                            vtw[:, 0, half * 512 : (half + 1) * 512],
                            start=True, stop=False,
                        )
                        nc.tensor.matmul(
                            po, mps[1][:, r0:r1],
                            vtw[:, 1, half * 512 : (half + 1) * 512],
                            start=False, stop=True,
                        )
                        dst = so[:, half * 512 : (half + 1) * 512]
                        if half == 0:
                            nc.vector.tensor_copy(dst, po)
                        else:
                            nc.scalar.activation(out=dst, in_=po, func=AF.Copy)
                    nc.sync.dma_start(out=out_d[r0:r1, n : n + 1024], in_=so)

    if split_waits:
        _split_excess_waits(nc)
    return nc


def _get_program():
    global _PROG
    if _PROG is None:
        _PROG = _build_program()
    return _PROG


def kernel(x, y, q_w, q_dw_w, kv_w, kv_dw_w, proj_w, temperature):
    return _run(x, y, q_w, q_dw_w, kv_w, kv_dw_w, proj_w, temperature)[0]


def _fold192(a):
    """[B, 192, HW] -> [B, 128, HW*1.5]: channels 128..191 packed beside the
    main block per 1024-pixel slab, halves stacked on partitions 0:64/64:128,
    so each input slab is ONE dma_start instead of two."""
    nsl = HW // 1024
    v = a.reshape(a.shape[0], C, nsl, 1024)
    out = np.empty((a.shape[0], 128, nsl, 1536), a.dtype)
    out[:, :, :, 0:1024] = v[:, 0:128]
    out[:, 0:64, :, 1024:1536] = v[:, 128:192, :, 0:512]
    out[:, 64:128, :, 1024:1536] = v[:, 128:192, :, 512:1024]
    return np.ascontiguousarray(out.reshape(a.shape[0], 128, nsl * 1536))


def _prepare_inputs(x, y, q_w, q_dw_w, kv_w, kv_dw_w, proj_w, temperature):
    x = np.asarray(x, dtype=np.float32).reshape(B, C, HW).astype(ml_dtypes.bfloat16)
    y = np.asarray(y, dtype=np.float32).reshape(B, C, HW).astype(ml_dtypes.bfloat16)
    x = _fold192(x)
    y = _fold192(y)
    q_w = np.asarray(q_w, dtype=np.float32)
    kv_w = np.asarray(kv_w, dtype=np.float32)
    proj_w = np.asarray(proj_w, dtype=np.float32)
    q_dw_w = np.asarray(q_dw_w, dtype=np.float32)
    kv_dw_w = np.asarray(kv_dw_w, dtype=np.float32)
    temperature = np.asarray(temperature, dtype=np.float32).reshape(HEADS)

    tq, _ = _build_toeplitz(q_dw_w[:, 0], 3)
    tq = tq.astype(ml_dtypes.float8_e3m4)
    tkv, skv = _build_toeplitz(kv_dw_w[:, 0], 7)
    tkv = tkv.astype(ml_dtypes.float8_e3m4)
    # interleave k and v tiles into 4-channel wave blocks:
    # block g = [k(4g..4g+3) x7 tiles | v(4g..4g+3) x7 tiles]
    tk = tkv[:, : C * 7].reshape(128, C // 4, 28, 128)
    tv = tkv[:, C * 7 :].reshape(128, C // 4, 28, 128)
    tkv2 = np.ascontiguousarray(
        np.concatenate([tk, tv], axis=2).reshape(128, 2 * C * 7, 128)
    )

    wq = q_w[:, :, 0, 0].T
    wkv = kv_w[:, :, 0, 0].T
    wqkv = np.concatenate([wq, wkv], axis=1)                # [C, C+384]
    # rows 128..191 duplicated onto both partition halves so either half of
    # the folded input slab finds aligned stationary weights
    wqkv = np.ascontiguousarray(
        np.concatenate([wqkv[0:128], wqkv[128:C], wqkv[128:C]], axis=0)
    ).astype(ml_dtypes.bfloat16)                            # [256, C+384]
    wpT = proj_w[:, :, 0, 0].T                              # [c_in, c_out]
    wp = np.ascontiguousarray(
        np.concatenate([wpT[0:PC], wpT[PC:C]], axis=1)
    ).astype(ml_dtypes.bfloat16)                            # [PC, 2C]
    # undo the per-channel fp8 amax scaling of v's depthwise weights on the
    # mp rows (indexed by v-channel d); q/k scales cancel in the l2 norm
    vsinv = (1.0 / skv[C:]).reshape(2, PC).T.astype(np.float32)  # [PC, 2]
    idb = np.eye(128, dtype=ml_dtypes.bfloat16)
    maskbd = np.zeros((PC, PC), np.float32)
    maskbd[0:DHC, 0:DHC] = 1.0
    maskbd[DHC:PC, DHC:PC] = 1.0
    tempcol = np.repeat(temperature, DHC).reshape(2, PC).T  # [PC, 2]
    maskv = np.ascontiguousarray(
        np.concatenate([maskbd, vsinv, tempcol], axis=1)
    ).astype(np.float32)

    shared = {
        "wqkv": wqkv, "wp": wp, "tq": tq, "tkv2": tkv2,
        "idb": idb, "maskv": maskv,
    }
    return [dict(shared, x=x[i], y=y[i]) for i in range(B)]


def _postprocess(out_core):
    return np.asarray(out_core).reshape(C, Himg, Wimg).astype(np.float32)


def _run(x, y, q_w, q_dw_w, kv_w, kv_dw_w, proj_w, temperature, trace=False):
    from concourse.bass_utils import run_bass_kernel_spmd

    in_maps = _prepare_inputs(
        x, y, q_w, q_dw_w, kv_w, kv_dw_w, proj_w, temperature
    )
    nc = _get_program()
    res = run_bass_kernel_spmd(
        nc, in_maps, core_ids=list(range(B)), trace=trace
    )
    out = np.stack([_postprocess(res.results[i]["out"]) for i in range(B)])
    return out, res

